# revision 5
# baseline (speedup 1.0000x reference)
"""BEiT-style windowed attention block on 8 Trainium2 NeuronCores.

Reference computation (per batch b, head h):
    qkv = x @ qkv_w.T + [q_bias, 0, v_bias]          # [B, N, 3C]
    q, k, v = split(qkv)                              # [B, H, N, D]
    s = (q * D**-0.5) @ k.T + rpb_table[rel_idx].T    # [B, H, N, N]
    p = softmax(s, axis=-1)
    out = (p @ v).reshape(B, N, C) @ proj_w.T + proj_b

Sharding: pure data parallel — batch 64 split as 8 batches per core,
weights + rel-pos-bias table replicated. No collectives.

Device-side layout strategy (per core):
  - x is staged host-side as x^T ("f-major": feature on partitions) so the
    qkv matmuls can use it as the moving operand directly.
  - q^T, k^T are produced f-major ([feat, token]) so the per-head attention
    matmul s^T[m, n] = k^T.T @ q^T needs no transposes.  Softmax runs over
    the partition (m) axis: exp on ACT, denominators via ones-column
    matmuls on the PE, division via a reciprocal row broadcast (DRAM-bounce
    DMA) — softmax is shift-invariant and the scores here are O(1), so the
    max-subtraction is skipped.
  - v is produced token-major ([token, feat]) which is exactly the lhsT
    layout stage-3 (p @ v) wants; its output comes out f-major, which is
    exactly the lhsT layout the final projection wants; the projection
    output comes out token-major, which is what the DMA back to HBM wants.
  - head pairs sit at partition offsets 0/64, so the K=64 / M=64 attention
    matmuls auto-pack into distinct PE row/col groups and run concurrently.
"""

import sys

sys.path.insert(0, "/opt/trn_rl_repo")

import numpy as np

import concourse.bass as bass
import concourse.mybir as mybir
import concourse.tile as tile
from concourse import bacc
from concourse.bass_utils import run_bass_kernel_spmd

F32 = mybir.dt.float32

DIM = 768
H = 12
D = 64
N = 197  # tokens per image
B = 64
CORES = 8
BSH = B // CORES  # batches per core
KO = DIM // 128  # contraction subtiles
SCALE = D ** -0.5
N0, N1 = 128, N - 128  # token chunk sizes (128, 69)


def build_program(n_batches: int = BSH):
    nc = bacc.Bacc("TRN2", target_bir_lowering=False, debug=False, num_devices=CORES)

    T = n_batches * N
    xt_d = nc.dram_tensor("xt", [128, KO, T], F32, kind="ExternalInput")
    qkw_d = nc.dram_tensor("qkw", [12, 128, KO, 128], F32, kind="ExternalInput")
    vw_d = nc.dram_tensor("vw", [128, KO, DIM], F32, kind="ExternalInput")
    pw_d = nc.dram_tensor("pw", [128, KO, DIM], F32, kind="ExternalInput")
    bias_d = nc.dram_tensor("bias", [2, 128, H, N], F32, kind="ExternalInput")
    qb_d = nc.dram_tensor("qb", [128, 12], F32, kind="ExternalInput")
    vb_d = nc.dram_tensor("vb", [1, DIM], F32, kind="ExternalInput")
    pb_d = nc.dram_tensor("pb", [1, DIM], F32, kind="ExternalInput")
    out_d = nc.dram_tensor("out", [n_batches, N, DIM], F32, kind="ExternalOutput")

    with tile.TileContext(nc) as tc:
        with (
            tc.tile_pool(name="wpool", bufs=1) as wpool,
            tc.tile_pool(name="xpool", bufs=2) as xpool,
            tc.tile_pool(name="qkpool", bufs=2) as qkpool,
            tc.tile_pool(name="vpool", bufs=2) as vpool,
            tc.tile_pool(name="spool", bufs=3) as spool,
            tc.tile_pool(name="epool", bufs=3) as epool,
            tc.tile_pool(name="opool", bufs=2) as opool,
            tc.tile_pool(name="otpool", bufs=3) as otpool,
            tc.tile_pool(name="outpool", bufs=2) as outpool,
            tc.tile_pool(name="ps_mm", bufs=3, space="PSUM") as ps_mm,
            tc.tile_pool(name="ps_s", bufs=3, space="PSUM") as ps_s,
            tc.tile_pool(name="ps_pd", bufs=2, space="PSUM") as ps_pd,
        ):
            # ---- persistent weights ----
            qkw = []
            for ft in range(12):
                t = wpool.tile([128, KO, 128], F32, tag=f"qkw{ft}")
                nc.sync.dma_start(t[:], qkw_d[ft])
                qkw.append(t)
            vw = wpool.tile([128, KO, DIM], F32, tag="vw")
            nc.sync.dma_start(vw[:], vw_d[:])
            pw = wpool.tile([128, KO, DIM], F32, tag="pw")
            nc.sync.dma_start(pw[:], pw_d[:])
            bias = wpool.tile([128, 2, H, N], F32, tag="bias")
            for mo in range(2):
                nc.sync.dma_start(bias[:, mo], bias_d[mo])
            qb = wpool.tile([128, 12], F32, tag="qb")
            nc.sync.dma_start(qb[:], qb_d[:])
            vb = wpool.tile([1, DIM], F32, tag="vb")
            nc.sync.dma_start(vb[:], vb_d[:])
            pb = wpool.tile([1, DIM], F32, tag="pb")
            nc.sync.dma_start(pb[:], pb_d[:])
            ones_wide = wpool.tile([128, 64], F32, tag="ones_wide")
            nc.vector.memset(ones_wide[:], 1.0)
            ones_row = wpool.tile([1, 128], F32, tag="ones_row")
            nc.vector.memset(ones_row[:], 1.0)

            assert n_batches % 2 == 0
            for chunk in range(n_batches // 2):
                # ---- load x^T for a 2-batch chunk ----
                xt = xpool.tile([128, KO, 2 * N], F32, tag="xt")
                nc.sync.dma_start(xt[:], xt_d[:, :, 2 * N * chunk : 2 * N * (chunk + 1)])

                # ---- q^T / k^T (f-major), both batches at once (N=394) ----
                qkT = [
                    qkpool.tile([128, 12, N], F32, tag=f"qkT{i}", name=f"qkT{i}") for i in range(2)
                ]
                for ft in range(12):
                    ps = ps_mm.tile([128, 512], F32, tag="mm")
                    for ko in range(KO):
                        nc.tensor.matmul(
                            ps[:, 0 : 2 * N],
                            qkw[ft][:, ko],
                            xt[:, ko],
                            start=(ko == 0),
                            stop=(ko == KO - 1),
                        )
                    for i in range(2):
                        nc.scalar.activation(
                            qkT[i][:, ft],
                            ps[:, i * N : (i + 1) * N],
                            mybir.ActivationFunctionType.Identity,
                            bias=qb[:, ft : ft + 1],
                            scale=SCALE if ft < 6 else 1.0,
                        )

                for i in range(2):
                    b = 2 * chunk + i
                    boff = i * N

                    # ---- v (token-major) ----
                    v_sb = vpool.tile([128, 2, H, D], F32, tag="v")
                    for no, tw in ((0, N0), (1, N1)):
                        for fo, fw in ((0, 512), (512, 256)):
                            psv = ps_mm.tile([128, 512], F32, tag="mm")
                            for ko in range(KO):
                                nc.tensor.matmul(
                                    psv[0:tw, 0:fw],
                                    xt[:, ko, boff + no * 128 : boff + no * 128 + tw],
                                    vw[:, ko, fo : fo + fw],
                                    start=(ko == 0),
                                    stop=False,
                                )
                            nc.tensor.matmul(
                                psv[0:tw, 0:fw],
                                ones_row[0:1, 0:tw],
                                vb[0:1, fo : fo + fw],
                                start=False,
                                stop=True,
                            )
                            nh = fw // D
                            nc.vector.tensor_copy(
                                v_sb[0:tw, no, fo // D : fo // D + nh, :],
                                psv[0:tw, 0:fw].rearrange("p (a b) -> p a b", a=nh),
                            )

                    # ---- attention, head pairs (2j, 2j+1) ----
                    ohT = opool.tile([128, KO, N], F32, tag="ohT")
                    for j in range(H // 2):
                        es_pair = []
                        for hh in range(2):
                            h = 2 * j + hh
                            base = (h % 2) * 64
                            fq = h // 2
                            fk = 6 + h // 2
                            pss = ps_s.tile([128, 512], F32, tag="s")
                            kT = qkT[i][base : base + 64, fk, :]
                            qT = qkT[i][base : base + 64, fq, :]
                            nc.tensor.matmul(
                                pss[:, 0:N], kT[:, 0:128], qT, start=True, stop=True
                            )
                            nc.tensor.matmul(
                                pss[0:N1, N : 2 * N], kT[:, 128:N], qT, start=True, stop=True
                            )
                            # bias add (DVE) then exp (ACT)
                            es = epool.tile([128, 2, N], F32, tag="es")
                            nc.vector.tensor_add(
                                es[:, 0, :], pss[:, 0:N], bias[:, 0, h, :]
                            )
                            nc.vector.tensor_add(
                                es[0:N1, 1, :], pss[0:N1, N : 2 * N], bias[0:N1, 1, h, :]
                            )
                            nc.scalar.activation(
                                es[:, 0, :], es[:, 0, :],
                                mybir.ActivationFunctionType.Exp,
                            )
                            nc.scalar.activation(
                                es[0:N1, 1, :], es[0:N1, 1, :],
                                mybir.ActivationFunctionType.Exp,
                            )
                            es_pair.append(es)

                        # stage 3: out^T[d, n] (cols 0:N) and denominators
                        # replicated across the 64 head dims (cols N:2N) so
                        # the division needs no partition broadcast.
                        pd = ps_pd.tile([128, 512], F32, tag="pd")
                        for hh in range(2):
                            h = 2 * j + hh
                            es = es_pair[hh]
                            rows = slice(hh * 64, hh * 64 + 64)
                            nc.tensor.matmul(
                                pd[rows, 0:N],
                                v_sb[:, 0, h, :],
                                es[:, 0, :],
                                start=True,
                                stop=False,
                            )
                            nc.tensor.matmul(
                                pd[rows, 0:N],
                                v_sb[0:N1, 1, h, :],
                                es[0:N1, 1, :],
                                start=False,
                                stop=True,
                            )
                            nc.tensor.matmul(
                                pd[rows, N : 2 * N],
                                ones_wide[:, :],
                                es[:, 0, :],
                                start=True,
                                stop=False,
                            )
                            nc.tensor.matmul(
                                pd[rows, N : 2 * N],
                                ones_wide[0:N1, :],
                                es[0:N1, 1, :],
                                start=False,
                                stop=True,
                            )

                        ot = otpool.tile([128, 2 * N], F32, tag="ot")
                        nc.scalar.activation(
                            ot[:], pd[:, 0 : 2 * N], mybir.ActivationFunctionType.Copy
                        )
                        rv = otpool.tile([128, N], F32, tag="rv")
                        nc.vector.reciprocal(rv[:], ot[:, N : 2 * N])
                        nc.vector.tensor_mul(ohT[:, j, :], ot[:, 0:N], rv[:])

                    # ---- projection (token-major out) + bias ----
                    out_sb = outpool.tile([128, 2, DIM], F32, tag="out")
                    for no, tw in ((0, N0), (1, N1)):
                        for fo, fw in ((0, 512), (512, 256)):
                            psp = ps_mm.tile([128, 512], F32, tag="mm")
                            for ko in range(KO):
                                nc.tensor.matmul(
                                    psp[0:tw, 0:fw],
                                    ohT[:, ko, no * 128 : no * 128 + tw],
                                    pw[:, ko, fo : fo + fw],
                                    start=(ko == 0),
                                    stop=False,
                                )
                            nc.tensor.matmul(
                                psp[0:tw, 0:fw],
                                ones_row[0:1, 0:tw],
                                pb[0:1, fo : fo + fw],
                                start=False,
                                stop=True,
                            )
                            nc.scalar.activation(
                                out_sb[0:tw, no, fo : fo + fw],
                                psp[0:tw, 0:fw],
                                mybir.ActivationFunctionType.Copy,
                            )
                    nc.sync.dma_start(out_d[b, 0:128, :], out_sb[:, 0, :])
                    nc.sync.dma_start(out_d[b, 128:N, :], out_sb[0:N1, 1, :])

    nc.compile()
    return nc


def prep_inputs(x, qkv_w, q_bias, v_bias, rpb_table, proj_w, proj_b, rel_idx):
    """Host-side staging: shard x over cores, lay out weights for SBUF."""
    x = np.asarray(x, dtype=np.float32)
    qkv_w = np.asarray(qkv_w, dtype=np.float32)
    proj_w = np.asarray(proj_w, dtype=np.float32)
    q_bias = np.asarray(q_bias, dtype=np.float32)
    v_bias = np.asarray(v_bias, dtype=np.float32)
    rpb_table = np.asarray(rpb_table, dtype=np.float32)
    proj_b = np.asarray(proj_b, dtype=np.float32)
    rel_idx = np.asarray(rel_idx)

    qkvwT = np.ascontiguousarray(qkv_w.T)  # [768, 2304]
    qkw = np.ascontiguousarray(
        qkvwT[:, : 2 * DIM].reshape(KO, 128, 12, 128).transpose(2, 1, 0, 3)
    )
    vw = np.ascontiguousarray(
        qkvwT[:, 2 * DIM :].reshape(KO, 128, DIM).transpose(1, 0, 2)
    )
    pw = np.ascontiguousarray(proj_w.T.reshape(KO, 128, DIM).transpose(1, 0, 2))

    # bias[mo, mi, h, n] = rpb_table[rel_idx[n, m], h] with m = mo*128 + mi
    bnm = rpb_table[rel_idx]  # [n, m, H]
    bias = np.zeros((2 * 128, H, N), dtype=np.float32)
    bias[:N] = bnm.transpose(1, 2, 0)  # [m, H, n]
    bias = bias.reshape(2, 128, H, N)

    qb = np.zeros((128, 12), dtype=np.float32)
    qb[:, :6] = (q_bias * SCALE).reshape(KO, 128).T
    vb = np.ascontiguousarray(v_bias[None, :])
    pb = np.ascontiguousarray(proj_b[None, :])

    shared = {
        "qkw": qkw, "vw": vw, "pw": pw, "bias": np.ascontiguousarray(bias),
        "qb": qb, "vb": vb, "pb": pb,
    }
    in_maps = []
    for c in range(CORES):
        xs = x[c * BSH : (c + 1) * BSH]  # [BSH, N, DIM]
        xt = np.ascontiguousarray(
            xs.reshape(BSH * N, DIM).T.reshape(KO, 128, BSH * N).transpose(1, 0, 2)
        )
        in_maps.append({"xt": xt, **shared})
    return in_maps


def _ensure_ntff_hook():
    """Register the axon NTFF profile hook so trace=True yields exec_time_ns.

    The image's antenv package lacks axon_hooks, so boot() degrades silently;
    supply the module via sys.modules and re-register the ctypes hook.
    Best-effort: failure only disables tracing, not execution."""
    import types

    if "antenv.axon_hooks" in sys.modules:
        return
    try:
        mod = types.ModuleType("antenv.axon_hooks")
        _hook = [None]
        mod.set_axon_ntff_profile_hook = lambda h: _hook.__setitem__(0, h)
        mod.get_axon_ntff_profile_hook = lambda: _hook[0]
        from trn_agent_boot.trn_boot import _ntff_profile_via_ctypes

        mod.set_axon_ntff_profile_hook(
            _ntff_profile_via_ctypes("/opt/axon/libaxon_pjrt.so")
        )
        sys.modules["antenv.axon_hooks"] = mod
    except Exception:
        pass


_NC = None


def _get_nc():
    global _NC
    if _NC is None:
        _NC = build_program(BSH)
    return _NC


def kernel(x, qkv_w, q_bias, v_bias, rpb_table, proj_w, proj_b, rel_idx,
           _trace=False, **trace_kwargs):
    if _trace:
        _ensure_ntff_hook()
    nc = _get_nc()
    in_maps = prep_inputs(x, qkv_w, q_bias, v_bias, rpb_table, proj_w, proj_b, rel_idx)
    res = run_bass_kernel_spmd(
        nc, in_maps, core_ids=list(range(CORES)), trace=_trace, **trace_kwargs
    )
    out = np.concatenate([res.results[c]["out"] for c in range(CORES)], axis=0)
    if _trace:
        return out, res
    return out


# revision 7
# speedup vs baseline: 1.9976x; 1.9976x over previous
"""BEiT-style windowed attention block on 8 Trainium2 NeuronCores.

Reference computation (per batch b, head h):
    qkv = x @ qkv_w.T + [q_bias, 0, v_bias]          # [B, N, 3C]
    q, k, v = split(qkv)                              # [B, H, N, D]
    s = (q * D**-0.5) @ k.T + rpb_table[rel_idx].T    # [B, H, N, N]
    p = softmax(s, axis=-1)
    out = (p @ v).reshape(B, N, C) @ proj_w.T + proj_b

Sharding: pure data parallel — batch 64 split as 8 batches per core,
weights + rel-pos-bias table replicated. No collectives.

Device-side layout strategy (per core):
  - x is staged host-side as x^T ("f-major": feature on partitions) so the
    qkv matmuls can use it as the moving operand directly.
  - q^T, k^T are produced f-major ([feat, token]) so the per-head attention
    matmul s^T[m, n] = k^T.T @ q^T needs no transposes.  Softmax runs over
    the partition (m) axis: exp on ACT, denominators via ones-column
    matmuls on the PE, division via a reciprocal row broadcast (DRAM-bounce
    DMA) — softmax is shift-invariant and the scores here are O(1), so the
    max-subtraction is skipped.
  - v is produced token-major ([token, feat]) which is exactly the lhsT
    layout stage-3 (p @ v) wants; its output comes out f-major, which is
    exactly the lhsT layout the final projection wants; the projection
    output comes out token-major, which is what the DMA back to HBM wants.
  - head pairs sit at partition offsets 0/64, so the K=64 / M=64 attention
    matmuls auto-pack into distinct PE row/col groups and run concurrently.
"""

import sys

sys.path.insert(0, "/opt/trn_rl_repo")

import numpy as np

import concourse.bass as bass
import concourse.mybir as mybir
import concourse.tile as tile
from concourse import bacc
from concourse.bass_utils import run_bass_kernel_spmd

F32 = mybir.dt.float32
# Matmul operand dtypes. float32r streams at full PE rate (vs plain fp32's
# LOW_HIGH double-pass at quarter rate) with ~1e-4 matmul error; bf16 is used
# for the small-N attention matmuls where float32r's 4-byte weight-load path
# dominates. PSUM accumulation and all softmax arithmetic stay fp32.
DT_BIG = mybir.dt.float32r
DT_ATT = mybir.dt.bfloat16

DIM = 768
H = 12
D = 64
N = 197  # tokens per image
B = 64
CORES = 8
BSH = B // CORES  # batches per core
KO = DIM // 128  # contraction subtiles
SCALE = D ** -0.5
N0, N1 = 128, N - 128  # token chunk sizes (128, 69)


def build_program(n_batches: int = BSH):
    nc = bacc.Bacc("TRN2", target_bir_lowering=False, debug=False, num_devices=CORES)

    T = n_batches * N
    xt_d = nc.dram_tensor("xt", [128, KO, T], DT_BIG, kind="ExternalInput")
    qkw_d = nc.dram_tensor("qkw", [12, 128, KO, 128], DT_BIG, kind="ExternalInput")
    vw_d = nc.dram_tensor("vw", [128, KO, DIM], DT_BIG, kind="ExternalInput")
    pw_d = nc.dram_tensor("pw", [128, KO, DIM], DT_BIG, kind="ExternalInput")
    bias_d = nc.dram_tensor("bias", [2, 128, H, N], F32, kind="ExternalInput")
    qb_d = nc.dram_tensor("qb", [128, 12], F32, kind="ExternalInput")
    vb_d = nc.dram_tensor("vb", [1, DIM], DT_BIG, kind="ExternalInput")
    pb_d = nc.dram_tensor("pb", [1, DIM], DT_BIG, kind="ExternalInput")
    onesr_d = nc.dram_tensor("onesr", [1, 128], DT_BIG, kind="ExternalInput")
    onesw_d = nc.dram_tensor("onesw", [128, 64], DT_ATT, kind="ExternalInput")
    out_d = nc.dram_tensor("out", [n_batches, N, DIM], F32, kind="ExternalOutput")

    with tile.TileContext(nc) as tc:
        with (
            tc.tile_pool(name="wpool", bufs=1) as wpool,
            tc.tile_pool(name="xpool", bufs=2) as xpool,
            tc.tile_pool(name="qkpool", bufs=2) as qkpool,
            tc.tile_pool(name="vpool", bufs=2) as vpool,
            tc.tile_pool(name="spool", bufs=3) as spool,
            tc.tile_pool(name="epool", bufs=3) as epool,
            tc.tile_pool(name="opool", bufs=2) as opool,
            tc.tile_pool(name="otpool", bufs=3) as otpool,
            tc.tile_pool(name="outpool", bufs=2) as outpool,
            tc.tile_pool(name="ps_mm", bufs=3, space="PSUM") as ps_mm,
            tc.tile_pool(name="ps_s", bufs=3, space="PSUM") as ps_s,
            tc.tile_pool(name="ps_pd", bufs=2, space="PSUM") as ps_pd,
        ):
            # ---- persistent weights ----
            qkw = []
            for ft in range(12):
                t = wpool.tile([128, KO, 128], DT_BIG, tag=f"qkw{ft}")
                nc.sync.dma_start(t[:], qkw_d[ft])
                qkw.append(t)
            vw = wpool.tile([128, KO, DIM], DT_BIG, tag="vw")
            nc.sync.dma_start(vw[:], vw_d[:])
            pw = wpool.tile([128, KO, DIM], DT_BIG, tag="pw")
            nc.sync.dma_start(pw[:], pw_d[:])
            bias = wpool.tile([128, 2, H, N], F32, tag="bias")
            for mo in range(2):
                nc.sync.dma_start(bias[:, mo], bias_d[mo])
            qb = wpool.tile([128, 12], F32, tag="qb")
            nc.sync.dma_start(qb[:], qb_d[:])
            vb = wpool.tile([1, DIM], DT_BIG, tag="vb")
            nc.sync.dma_start(vb[:], vb_d[:])
            pb = wpool.tile([1, DIM], DT_BIG, tag="pb")
            nc.sync.dma_start(pb[:], pb_d[:])
            ones_wide = wpool.tile([128, 64], DT_ATT, tag="ones_wide")
            nc.sync.dma_start(ones_wide[:], onesw_d[:])
            ones_row = wpool.tile([1, 128], DT_BIG, tag="ones_row")
            nc.sync.dma_start(ones_row[:], onesr_d[:])

            assert n_batches % 2 == 0
            for chunk in range(n_batches // 2):
                # ---- load x^T for a 2-batch chunk ----
                xt = xpool.tile([128, KO, 2 * N], DT_BIG, tag="xt")
                nc.sync.dma_start(xt[:], xt_d[:, :, 2 * N * chunk : 2 * N * (chunk + 1)])

                # ---- q^T / k^T (f-major), both batches at once (N=394) ----
                qkT = [
                    qkpool.tile([128, 12, N], DT_ATT, tag=f"qkT{i}", name=f"qkT{i}") for i in range(2)
                ]
                for ft in range(12):
                    ps = ps_mm.tile([128, 512], F32, tag="mm")
                    for ko in range(KO):
                        nc.tensor.matmul(
                            ps[:, 0 : 2 * N],
                            qkw[ft][:, ko],
                            xt[:, ko],
                            start=(ko == 0),
                            stop=(ko == KO - 1),
                        )
                    for i in range(2):
                        nc.scalar.activation(
                            qkT[i][:, ft],
                            ps[:, i * N : (i + 1) * N],
                            mybir.ActivationFunctionType.Identity,
                            bias=qb[:, ft : ft + 1],
                            scale=SCALE if ft < 6 else 1.0,
                        )

                for i in range(2):
                    b = 2 * chunk + i
                    boff = i * N

                    # ---- v (token-major) ----
                    v_sb = vpool.tile([128, 2, H, D], DT_ATT, tag="v")
                    for no, tw in ((0, N0), (1, N1)):
                        for fo, fw in ((0, 512), (512, 256)):
                            psv = ps_mm.tile([128, 512], F32, tag="mm")
                            for ko in range(KO):
                                nc.tensor.matmul(
                                    psv[0:tw, 0:fw],
                                    xt[:, ko, boff + no * 128 : boff + no * 128 + tw],
                                    vw[:, ko, fo : fo + fw],
                                    start=(ko == 0),
                                    stop=False,
                                )
                            nc.tensor.matmul(
                                psv[0:tw, 0:fw],
                                ones_row[0:1, 0:tw],
                                vb[0:1, fo : fo + fw],
                                start=False,
                                stop=True,
                            )
                            nh = fw // D
                            nc.vector.tensor_copy(
                                v_sb[0:tw, no, fo // D : fo // D + nh, :],
                                psv[0:tw, 0:fw].rearrange("p (a b) -> p a b", a=nh),
                            )

                    # ---- attention, head pairs (2j, 2j+1) ----
                    ohT = opool.tile([128, KO, N], DT_BIG, tag="ohT")
                    for j in range(H // 2):
                        es_pair = []
                        for hh in range(2):
                            h = 2 * j + hh
                            base = (h % 2) * 64
                            fq = h // 2
                            fk = 6 + h // 2
                            pss = ps_s.tile([128, 512], F32, tag="s")
                            kT = qkT[i][base : base + 64, fk, :]
                            qT = qkT[i][base : base + 64, fq, :]
                            nc.tensor.matmul(
                                pss[:, 0:N], kT[:, 0:128], qT, start=True, stop=True
                            )
                            nc.tensor.matmul(
                                pss[0:N1, N : 2 * N], kT[:, 128:N], qT, start=True, stop=True
                            )
                            # bias add (DVE, fp32) then exp (ACT, -> DT_ATT)
                            stmp = spool.tile([128, 2, N], F32, tag="stmp")
                            es = epool.tile([128, 2, N], DT_ATT, tag="es")
                            nc.vector.tensor_add(
                                stmp[:, 0, :], pss[:, 0:N], bias[:, 0, h, :]
                            )
                            nc.vector.tensor_add(
                                stmp[0:N1, 1, :], pss[0:N1, N : 2 * N], bias[0:N1, 1, h, :]
                            )
                            nc.scalar.activation(
                                es[:, 0, :], stmp[:, 0, :],
                                mybir.ActivationFunctionType.Exp,
                            )
                            nc.scalar.activation(
                                es[0:N1, 1, :], stmp[0:N1, 1, :],
                                mybir.ActivationFunctionType.Exp,
                            )
                            es_pair.append(es)

                        # stage 3: out^T[d, n] (cols 0:N) and denominators
                        # replicated across the 64 head dims (cols N:2N) so
                        # the division needs no partition broadcast.
                        pd = ps_pd.tile([128, 512], F32, tag="pd")
                        for hh in range(2):
                            h = 2 * j + hh
                            es = es_pair[hh]
                            rows = slice(hh * 64, hh * 64 + 64)
                            nc.tensor.matmul(
                                pd[rows, 0:N],
                                v_sb[:, 0, h, :],
                                es[:, 0, :],
                                start=True,
                                stop=False,
                            )
                            nc.tensor.matmul(
                                pd[rows, 0:N],
                                v_sb[0:N1, 1, h, :],
                                es[0:N1, 1, :],
                                start=False,
                                stop=True,
                            )
                            nc.tensor.matmul(
                                pd[rows, N : 2 * N],
                                ones_wide[:, :],
                                es[:, 0, :],
                                start=True,
                                stop=False,
                            )
                            nc.tensor.matmul(
                                pd[rows, N : 2 * N],
                                ones_wide[0:N1, :],
                                es[0:N1, 1, :],
                                start=False,
                                stop=True,
                            )

                        ot = otpool.tile([128, 2 * N], F32, tag="ot")
                        nc.scalar.activation(
                            ot[:], pd[:, 0 : 2 * N], mybir.ActivationFunctionType.Copy
                        )
                        rv = otpool.tile([128, N], F32, tag="rv")
                        nc.vector.reciprocal(rv[:], ot[:, N : 2 * N])
                        nc.vector.tensor_mul(ohT[:, j, :], ot[:, 0:N], rv[:])

                    # ---- projection (token-major out) + bias ----
                    out_sb = outpool.tile([128, 2, DIM], F32, tag="out")
                    for no, tw in ((0, N0), (1, N1)):
                        for fo, fw in ((0, 512), (512, 256)):
                            psp = ps_mm.tile([128, 512], F32, tag="mm")
                            for ko in range(KO):
                                nc.tensor.matmul(
                                    psp[0:tw, 0:fw],
                                    ohT[:, ko, no * 128 : no * 128 + tw],
                                    pw[:, ko, fo : fo + fw],
                                    start=(ko == 0),
                                    stop=False,
                                )
                            nc.tensor.matmul(
                                psp[0:tw, 0:fw],
                                ones_row[0:1, 0:tw],
                                pb[0:1, fo : fo + fw],
                                start=False,
                                stop=True,
                            )
                            nc.scalar.activation(
                                out_sb[0:tw, no, fo : fo + fw],
                                psp[0:tw, 0:fw],
                                mybir.ActivationFunctionType.Copy,
                            )
                    nc.sync.dma_start(out_d[b, 0:128, :], out_sb[:, 0, :])
                    nc.sync.dma_start(out_d[b, 128:N, :], out_sb[0:N1, 1, :])

    nc.compile()
    return nc


def prep_inputs(x, qkv_w, q_bias, v_bias, rpb_table, proj_w, proj_b, rel_idx):
    """Host-side staging: shard x over cores, lay out weights for SBUF."""
    x = np.asarray(x, dtype=np.float32)
    qkv_w = np.asarray(qkv_w, dtype=np.float32)
    proj_w = np.asarray(proj_w, dtype=np.float32)
    q_bias = np.asarray(q_bias, dtype=np.float32)
    v_bias = np.asarray(v_bias, dtype=np.float32)
    rpb_table = np.asarray(rpb_table, dtype=np.float32)
    proj_b = np.asarray(proj_b, dtype=np.float32)
    rel_idx = np.asarray(rel_idx)

    qkvwT = np.ascontiguousarray(qkv_w.T)  # [768, 2304]
    qkw = np.ascontiguousarray(
        qkvwT[:, : 2 * DIM].reshape(KO, 128, 12, 128).transpose(2, 1, 0, 3)
    )
    vw = np.ascontiguousarray(
        qkvwT[:, 2 * DIM :].reshape(KO, 128, DIM).transpose(1, 0, 2)
    )
    pw = np.ascontiguousarray(proj_w.T.reshape(KO, 128, DIM).transpose(1, 0, 2))

    # bias[mo, mi, h, n] = rpb_table[rel_idx[n, m], h] with m = mo*128 + mi
    bnm = rpb_table[rel_idx]  # [n, m, H]
    bias = np.zeros((2 * 128, H, N), dtype=np.float32)
    bias[:N] = bnm.transpose(1, 2, 0)  # [m, H, n]
    bias = bias.reshape(2, 128, H, N)

    qb = np.zeros((128, 12), dtype=np.float32)
    qb[:, :6] = (q_bias * SCALE).reshape(KO, 128).T
    vb = np.ascontiguousarray(v_bias[None, :])
    pb = np.ascontiguousarray(proj_b[None, :])

    import ml_dtypes

    shared = {
        "qkw": qkw, "vw": vw, "pw": pw, "bias": np.ascontiguousarray(bias),
        "qb": qb, "vb": vb, "pb": pb,
        "onesr": np.ones((1, 128), dtype=np.float32),
        "onesw": np.ones((128, 64), dtype=ml_dtypes.bfloat16),
    }
    in_maps = []
    for c in range(CORES):
        xs = x[c * BSH : (c + 1) * BSH]  # [BSH, N, DIM]
        xt = np.ascontiguousarray(
            xs.reshape(BSH * N, DIM).T.reshape(KO, 128, BSH * N).transpose(1, 0, 2)
        )
        in_maps.append({"xt": xt, **shared})
    return in_maps


def _ensure_ntff_hook():
    """Register the axon NTFF profile hook so trace=True yields exec_time_ns.

    The image's antenv package lacks axon_hooks, so boot() degrades silently;
    supply the module via sys.modules and re-register the ctypes hook.
    Best-effort: failure only disables tracing, not execution."""
    import types

    if "antenv.axon_hooks" in sys.modules:
        return
    try:
        mod = types.ModuleType("antenv.axon_hooks")
        _hook = [None]
        mod.set_axon_ntff_profile_hook = lambda h: _hook.__setitem__(0, h)
        mod.get_axon_ntff_profile_hook = lambda: _hook[0]
        from trn_agent_boot.trn_boot import _ntff_profile_via_ctypes

        mod.set_axon_ntff_profile_hook(
            _ntff_profile_via_ctypes("/opt/axon/libaxon_pjrt.so")
        )
        sys.modules["antenv.axon_hooks"] = mod
    except Exception:
        pass


_NC = None


def _get_nc():
    global _NC
    if _NC is None:
        _NC = build_program(BSH)
    return _NC


def kernel(x, qkv_w, q_bias, v_bias, rpb_table, proj_w, proj_b, rel_idx,
           _trace=False, **trace_kwargs):
    if _trace:
        _ensure_ntff_hook()
    nc = _get_nc()
    in_maps = prep_inputs(x, qkv_w, q_bias, v_bias, rpb_table, proj_w, proj_b, rel_idx)
    res = run_bass_kernel_spmd(
        nc, in_maps, core_ids=list(range(CORES)), trace=_trace, **trace_kwargs
    )
    out = np.concatenate([res.results[c]["out"] for c in range(CORES)], axis=0)
    if _trace:
        return out, res
    return out


# revision 9
# speedup vs baseline: 2.0336x; 1.0180x over previous
"""BEiT-style windowed attention block on 8 Trainium2 NeuronCores.

Reference computation (per batch b, head h):
    qkv = x @ qkv_w.T + [q_bias, 0, v_bias]          # [B, N, 3C]
    q, k, v = split(qkv)                              # [B, H, N, D]
    s = (q * D**-0.5) @ k.T + rpb_table[rel_idx].T    # [B, H, N, N]
    p = softmax(s, axis=-1)
    out = (p @ v).reshape(B, N, C) @ proj_w.T + proj_b

Sharding: pure data parallel — batch 64 split as 8 batches per core,
weights + rel-pos-bias table replicated. No collectives.

Device-side layout strategy (per core):
  - x is staged host-side as x^T ("f-major": feature on partitions) so the
    qkv matmuls can use it as the moving operand directly.
  - q^T, k^T are produced f-major ([feat, token]) so the per-head attention
    matmul s^T[m, n] = k^T.T @ q^T needs no transposes.  Softmax runs over
    the partition (m) axis: exp on ACT, denominators via ones-column
    matmuls on the PE, division via a reciprocal row broadcast (DRAM-bounce
    DMA) — softmax is shift-invariant and the scores here are O(1), so the
    max-subtraction is skipped.
  - v is produced token-major ([token, feat]) which is exactly the lhsT
    layout stage-3 (p @ v) wants; its output comes out f-major, which is
    exactly the lhsT layout the final projection wants; the projection
    output comes out token-major, which is what the DMA back to HBM wants.
  - head pairs sit at partition offsets 0/64, so the K=64 / M=64 attention
    matmuls auto-pack into distinct PE row/col groups and run concurrently.
"""

import sys

sys.path.insert(0, "/opt/trn_rl_repo")

import numpy as np

import concourse.bass as bass
import concourse.mybir as mybir
import concourse.tile as tile
from concourse import bacc
from concourse.bass_utils import run_bass_kernel_spmd

F32 = mybir.dt.float32
# Matmul operand dtypes. float32r streams at full PE rate (vs plain fp32's
# LOW_HIGH double-pass at quarter rate) with ~1e-4 matmul error; bf16 is used
# for the small-N attention matmuls where float32r's 4-byte weight-load path
# dominates. PSUM accumulation and all softmax arithmetic stay fp32.
DT_BIG = mybir.dt.float32r
DT_ATT = mybir.dt.bfloat16

DIM = 768
H = 12
D = 64
N = 197  # tokens per image
B = 64
CORES = 8
BSH = B // CORES  # batches per core
KO = DIM // 128  # contraction subtiles
SCALE = D ** -0.5
N0, N1 = 128, N - 128  # token chunk sizes (128, 69)


def build_program(n_batches: int = BSH):
    nc = bacc.Bacc("TRN2", target_bir_lowering=False, debug=False, num_devices=CORES)

    T = n_batches * N
    xt_d = nc.dram_tensor("xt", [128, KO, T], DT_BIG, kind="ExternalInput")
    qkw_d = nc.dram_tensor("qkw", [12, 128, KO, 128], DT_BIG, kind="ExternalInput")
    vw_d = nc.dram_tensor("vw", [128, KO, DIM], DT_BIG, kind="ExternalInput")
    pw_d = nc.dram_tensor("pw", [128, KO, DIM], DT_BIG, kind="ExternalInput")
    bias_d = nc.dram_tensor("bias", [2, 128, H, N], F32, kind="ExternalInput")
    qb_d = nc.dram_tensor("qb", [128, 12], F32, kind="ExternalInput")
    vb_d = nc.dram_tensor("vb", [1, DIM], DT_BIG, kind="ExternalInput")
    pb_d = nc.dram_tensor("pb", [1, DIM], DT_BIG, kind="ExternalInput")
    onesr_d = nc.dram_tensor("onesr", [1, 128], DT_BIG, kind="ExternalInput")
    onesw_d = nc.dram_tensor("onesw", [128, 64], DT_ATT, kind="ExternalInput")
    out_d = nc.dram_tensor("out", [n_batches, N, DIM], F32, kind="ExternalOutput")

    with tile.TileContext(nc) as tc:
        with (
            tc.tile_pool(name="wpool", bufs=1) as wpool,
            tc.tile_pool(name="xpool", bufs=2) as xpool,
            tc.tile_pool(name="qkpool", bufs=2) as qkpool,
            tc.tile_pool(name="vpool", bufs=2) as vpool,
            tc.tile_pool(name="spool", bufs=3) as spool,
            tc.tile_pool(name="epool", bufs=3) as epool,
            tc.tile_pool(name="opool", bufs=2) as opool,
            tc.tile_pool(name="otpool", bufs=3) as otpool,
            tc.tile_pool(name="outpool", bufs=2) as outpool,
            tc.tile_pool(name="ps_mm", bufs=3, space="PSUM") as ps_mm,
            tc.tile_pool(name="ps_s", bufs=3, space="PSUM") as ps_s,
            tc.tile_pool(name="ps_pd", bufs=2, space="PSUM") as ps_pd,
        ):
            # ---- persistent weights ----
            qkw = []
            for ft in range(12):
                t = wpool.tile([128, KO, 128], DT_BIG, tag=f"qkw{ft}")
                nc.sync.dma_start(t[:], qkw_d[ft])
                qkw.append(t)
            vw = wpool.tile([128, KO, DIM], DT_BIG, tag="vw")
            nc.sync.dma_start(vw[:], vw_d[:])
            pw = wpool.tile([128, KO, DIM], DT_BIG, tag="pw")
            nc.sync.dma_start(pw[:], pw_d[:])
            bias = wpool.tile([128, 2, H, N], F32, tag="bias")
            for mo in range(2):
                nc.sync.dma_start(bias[:, mo], bias_d[mo])
            qb = wpool.tile([128, 12], F32, tag="qb")
            nc.sync.dma_start(qb[:], qb_d[:])
            vb = wpool.tile([1, DIM], DT_BIG, tag="vb")
            nc.sync.dma_start(vb[:], vb_d[:])
            pb = wpool.tile([1, DIM], DT_BIG, tag="pb")
            nc.sync.dma_start(pb[:], pb_d[:])
            ones_wide = wpool.tile([128, 64], DT_ATT, tag="ones_wide")
            nc.sync.dma_start(ones_wide[:], onesw_d[:])
            ones_row = wpool.tile([1, 128], DT_BIG, tag="ones_row")
            nc.sync.dma_start(ones_row[:], onesr_d[:])

            assert n_batches % 2 == 0
            for chunk in range(n_batches // 2):
                # ---- load x^T for a 2-batch chunk ----
                xt = xpool.tile([128, KO, 2 * N], DT_BIG, tag="xt")
                nc.sync.dma_start(xt[:], xt_d[:, :, 2 * N * chunk : 2 * N * (chunk + 1)])

                # ---- q^T / k^T (f-major), both batches at once (N=394) ----
                qkT = [
                    qkpool.tile([128, 12, N], DT_ATT, tag=f"qkT{i}", name=f"qkT{i}") for i in range(2)
                ]
                for ft in range(12):
                    ps = ps_mm.tile([128, 512], F32, tag="mm")
                    for ko in range(KO):
                        nc.tensor.matmul(
                            ps[:, 0 : 2 * N],
                            qkw[ft][:, ko],
                            xt[:, ko],
                            start=(ko == 0),
                            stop=(ko == KO - 1),
                        )
                    for i in range(2):
                        nc.scalar.activation(
                            qkT[i][:, ft],
                            ps[:, i * N : (i + 1) * N],
                            mybir.ActivationFunctionType.Identity,
                            bias=qb[:, ft : ft + 1],
                            scale=SCALE if ft < 6 else 1.0,
                        )

                for i in range(2):
                    b = 2 * chunk + i
                    boff = i * N

                    # ---- v (token-major) ----
                    v_sb = vpool.tile([128, 2, H, D], DT_ATT, tag="v")
                    for no, tw in ((0, N0), (1, N1)):
                        for fo, fw in ((0, 512), (512, 256)):
                            psv = ps_mm.tile([128, 512], F32, tag="mm")
                            for ko in range(KO):
                                nc.tensor.matmul(
                                    psv[0:tw, 0:fw],
                                    xt[:, ko, boff + no * 128 : boff + no * 128 + tw],
                                    vw[:, ko, fo : fo + fw],
                                    start=(ko == 0),
                                    stop=False,
                                )
                            nc.tensor.matmul(
                                psv[0:tw, 0:fw],
                                ones_row[0:1, 0:tw],
                                vb[0:1, fo : fo + fw],
                                start=False,
                                stop=True,
                            )
                            nh = fw // D
                            nc.vector.tensor_copy(
                                v_sb[0:tw, no, fo // D : fo // D + nh, :],
                                psv[0:tw, 0:fw].rearrange("p (a b) -> p a b", a=nh),
                            )

                    # ---- attention, head pairs (2j, 2j+1) ----
                    ohT = opool.tile([128, KO, N], DT_BIG, tag="ohT")
                    for j in range(H // 2):
                        es_pair = []
                        for hh in range(2):
                            h = 2 * j + hh
                            base = (h % 2) * 64
                            fq = h // 2
                            fk = 6 + h // 2
                            pss = ps_s.tile([128, 512], F32, tag="s")
                            kT = qkT[i][base : base + 64, fk, :]
                            qT = qkT[i][base : base + 64, fq, :]
                            nc.tensor.matmul(
                                pss[:, 0:N], kT[:, 0:128], qT, start=True, stop=True
                            )
                            nc.tensor.matmul(
                                pss[0:N1, N : 2 * N], kT[:, 128:N], qT, start=True, stop=True
                            )
                            # bias add (DVE, fp32) then exp (ACT, -> DT_ATT)
                            stmp = spool.tile([128, 2, N], F32, tag="stmp")
                            es = epool.tile([128, 2, N], DT_ATT, tag="es")
                            nc.vector.tensor_add(
                                stmp[:, 0, :], pss[:, 0:N], bias[:, 0, h, :]
                            )
                            nc.vector.tensor_add(
                                stmp[0:N1, 1, :], pss[0:N1, N : 2 * N], bias[0:N1, 1, h, :]
                            )
                            nc.scalar.activation(
                                es[:, 0, :], stmp[:, 0, :],
                                mybir.ActivationFunctionType.Exp,
                            )
                            nc.scalar.activation(
                                es[0:N1, 1, :], stmp[0:N1, 1, :],
                                mybir.ActivationFunctionType.Exp,
                            )
                            es_pair.append(es)

                        # stage 3: out^T[d, n] (cols 0:N) and denominators
                        # replicated across the 64 head dims (cols N:2N) so
                        # the division needs no partition broadcast.
                        pd = ps_pd.tile([128, 512], F32, tag="pd")
                        for hh in range(2):
                            h = 2 * j + hh
                            es = es_pair[hh]
                            rows = slice(hh * 64, hh * 64 + 64)
                            nc.tensor.matmul(
                                pd[rows, 0:N],
                                v_sb[:, 0, h, :],
                                es[:, 0, :],
                                start=True,
                                stop=False,
                            )
                            nc.tensor.matmul(
                                pd[rows, 0:N],
                                v_sb[0:N1, 1, h, :],
                                es[0:N1, 1, :],
                                start=False,
                                stop=True,
                            )
                            nc.tensor.matmul(
                                pd[rows, N : 2 * N],
                                ones_wide[:, :],
                                es[:, 0, :],
                                start=True,
                                stop=False,
                            )
                            nc.tensor.matmul(
                                pd[rows, N : 2 * N],
                                ones_wide[0:N1, :],
                                es[0:N1, 1, :],
                                start=False,
                                stop=True,
                            )

                        ot = otpool.tile([128, 2 * N], F32, tag="ot")
                        nc.scalar.activation(
                            ot[:], pd[:, 0 : 2 * N], mybir.ActivationFunctionType.Copy
                        )
                        rv = otpool.tile([128, N], F32, tag="rv")
                        nc.vector.reciprocal(rv[:], ot[:, N : 2 * N])
                        nc.vector.tensor_mul(ohT[:, j, :], ot[:, 0:N], rv[:])

                    # ---- projection (token-major out) + bias ----
                    out_sb = outpool.tile([128, 2, DIM], F32, tag="out")
                    for no, tw in ((0, N0), (1, N1)):
                        for fo, fw in ((0, 512), (512, 256)):
                            psp = ps_mm.tile([128, 512], F32, tag="mm")
                            for ko in range(KO):
                                nc.tensor.matmul(
                                    psp[0:tw, 0:fw],
                                    ohT[:, ko, no * 128 : no * 128 + tw],
                                    pw[:, ko, fo : fo + fw],
                                    start=(ko == 0),
                                    stop=False,
                                )
                            nc.tensor.matmul(
                                psp[0:tw, 0:fw],
                                ones_row[0:1, 0:tw],
                                pb[0:1, fo : fo + fw],
                                start=False,
                                stop=True,
                            )
                            nc.scalar.activation(
                                out_sb[0:tw, no, fo : fo + fw],
                                psp[0:tw, 0:fw],
                                mybir.ActivationFunctionType.Copy,
                            )
                    nc.sync.dma_start(out_d[b, 0:128, :], out_sb[:, 0, :])
                    nc.sync.dma_start(out_d[b, 128:N, :], out_sb[0:N1, 1, :])

    nc.compile()
    return nc


def prep_inputs(x, qkv_w, q_bias, v_bias, rpb_table, proj_w, proj_b, rel_idx):
    """Host-side staging: shard x over cores, lay out weights for SBUF."""
    x = np.asarray(x, dtype=np.float32)
    qkv_w = np.asarray(qkv_w, dtype=np.float32)
    proj_w = np.asarray(proj_w, dtype=np.float32)
    q_bias = np.asarray(q_bias, dtype=np.float32)
    v_bias = np.asarray(v_bias, dtype=np.float32)
    rpb_table = np.asarray(rpb_table, dtype=np.float32)
    proj_b = np.asarray(proj_b, dtype=np.float32)
    rel_idx = np.asarray(rel_idx)

    qkvwT = np.ascontiguousarray(qkv_w.T)  # [768, 2304]
    qkw = np.ascontiguousarray(
        qkvwT[:, : 2 * DIM].reshape(KO, 128, 12, 128).transpose(2, 1, 0, 3)
    )
    vw = np.ascontiguousarray(
        qkvwT[:, 2 * DIM :].reshape(KO, 128, DIM).transpose(1, 0, 2)
    )
    pw = np.ascontiguousarray(proj_w.T.reshape(KO, 128, DIM).transpose(1, 0, 2))

    # bias[mo, mi, h, n] = rpb_table[rel_idx[n, m], h] with m = mo*128 + mi
    bnm = rpb_table[rel_idx]  # [n, m, H]
    bias = np.zeros((2 * 128, H, N), dtype=np.float32)
    bias[:N] = bnm.transpose(1, 2, 0)  # [m, H, n]
    bias = bias.reshape(2, 128, H, N)

    qb = np.zeros((128, 12), dtype=np.float32)
    qb[:, :6] = (q_bias * SCALE).reshape(KO, 128).T
    vb = np.ascontiguousarray(v_bias[None, :])
    pb = np.ascontiguousarray(proj_b[None, :])

    import ml_dtypes

    shared = {
        "qkw": qkw, "vw": vw, "pw": pw, "bias": np.ascontiguousarray(bias),
        "qb": qb, "vb": vb, "pb": pb,
        "onesr": np.ones((1, 128), dtype=np.float32),
        "onesw": np.ones((128, 64), dtype=np.float32 if DT_ATT != mybir.dt.bfloat16 else ml_dtypes.bfloat16),
    }
    in_maps = []
    for c in range(CORES):
        xs = x[c * BSH : (c + 1) * BSH]  # [BSH, N, DIM]
        xt = np.ascontiguousarray(
            xs.reshape(BSH * N, DIM).T.reshape(KO, 128, BSH * N).transpose(1, 0, 2)
        )
        in_maps.append({"xt": xt, **shared})
    return in_maps


def _ensure_ntff_hook():
    """Register the axon NTFF profile hook so trace=True yields exec_time_ns.

    The image's antenv package lacks axon_hooks, so boot() degrades silently;
    supply the module via sys.modules and re-register the ctypes hook.
    Best-effort: failure only disables tracing, not execution."""
    import types

    if "antenv.axon_hooks" in sys.modules:
        return
    try:
        mod = types.ModuleType("antenv.axon_hooks")
        _hook = [None]
        mod.set_axon_ntff_profile_hook = lambda h: _hook.__setitem__(0, h)
        mod.get_axon_ntff_profile_hook = lambda: _hook[0]
        from trn_agent_boot.trn_boot import _ntff_profile_via_ctypes

        mod.set_axon_ntff_profile_hook(
            _ntff_profile_via_ctypes("/opt/axon/libaxon_pjrt.so")
        )
        sys.modules["antenv.axon_hooks"] = mod
    except Exception:
        pass


_NC = None


def _get_nc():
    global _NC
    if _NC is None:
        _NC = build_program(BSH)
    return _NC


def kernel(x, qkv_w, q_bias, v_bias, rpb_table, proj_w, proj_b, rel_idx,
           _trace=False, **trace_kwargs):
    if _trace:
        _ensure_ntff_hook()
    nc = _get_nc()
    in_maps = prep_inputs(x, qkv_w, q_bias, v_bias, rpb_table, proj_w, proj_b, rel_idx)
    res = run_bass_kernel_spmd(
        nc, in_maps, core_ids=list(range(CORES)), trace=_trace, **trace_kwargs
    )
    out = np.concatenate([res.results[c]["out"] for c in range(CORES)], axis=0)
    if _trace:
        return out, res
    return out


# revision 10
# speedup vs baseline: 2.2748x; 1.1186x over previous
"""BEiT-style windowed attention block on 8 Trainium2 NeuronCores.

Reference computation (per batch b, head h):
    qkv = x @ qkv_w.T + [q_bias, 0, v_bias]          # [B, N, 3C]
    q, k, v = split(qkv)                              # [B, H, N, D]
    s = (q * D**-0.5) @ k.T + rpb_table[rel_idx].T    # [B, H, N, N]
    p = softmax(s, axis=-1)
    out = (p @ v).reshape(B, N, C) @ proj_w.T + proj_b

Sharding: pure data parallel — batch 64 split as 8 batches per core,
weights + rel-pos-bias table replicated. No collectives.

Device-side layout strategy (per core):
  - x is staged host-side as x^T ("f-major": feature on partitions) so the
    qkv matmuls can use it as the moving operand directly.
  - q^T, k^T are produced f-major ([feat, token]) so the per-head attention
    matmul s^T[m, n] = k^T.T @ q^T needs no transposes.  Softmax runs over
    the partition (m) axis: exp on ACT, denominators via ones-column
    matmuls on the PE, division via a reciprocal row broadcast (DRAM-bounce
    DMA) — softmax is shift-invariant and the scores here are O(1), so the
    max-subtraction is skipped.
  - v is produced token-major ([token, feat]) which is exactly the lhsT
    layout stage-3 (p @ v) wants; its output comes out f-major, which is
    exactly the lhsT layout the final projection wants; the projection
    output comes out token-major, which is what the DMA back to HBM wants.
  - head pairs sit at partition offsets 0/64, so the K=64 / M=64 attention
    matmuls auto-pack into distinct PE row/col groups and run concurrently.
"""

import sys

sys.path.insert(0, "/opt/trn_rl_repo")

import numpy as np

import concourse.bass as bass
import concourse.mybir as mybir
import concourse.tile as tile
from concourse import bacc
from concourse.bass_utils import run_bass_kernel_spmd

F32 = mybir.dt.float32
# Matmul operand dtype. fp16 streams at 1 row/cycle (4x plain fp32's LOW_HIGH
# double-pass), keeps the PE HAM clock warm (unlike float32r, whose datapath
# doesn't register as PE activity and re-throttles the clock to 1.2 GHz), and
# carries 3 more mantissa bits than bf16. All values here are O(100) at most,
# far from fp16 range limits. PSUM accumulation and softmax arithmetic stay
# fp32.
DT_BIG = mybir.dt.float16
DT_ATT = mybir.dt.float16

DIM = 768
H = 12
D = 64
N = 197  # tokens per image
B = 64
CORES = 8
BSH = B // CORES  # batches per core
KO = DIM // 128  # contraction subtiles
SCALE = D ** -0.5
N0, N1 = 128, N - 128  # token chunk sizes (128, 69)


def build_program(n_batches: int = BSH):
    nc = bacc.Bacc("TRN2", target_bir_lowering=False, debug=False, num_devices=CORES)

    T = n_batches * N
    xt_d = nc.dram_tensor("xt", [128, KO, T], DT_BIG, kind="ExternalInput")
    qkw_d = nc.dram_tensor("qkw", [12, 128, KO, 128], DT_BIG, kind="ExternalInput")
    vw_d = nc.dram_tensor("vw", [128, KO, DIM], DT_BIG, kind="ExternalInput")
    pw_d = nc.dram_tensor("pw", [128, KO, DIM], DT_BIG, kind="ExternalInput")
    bias_d = nc.dram_tensor("bias", [2, 128, H, N], F32, kind="ExternalInput")
    qb_d = nc.dram_tensor("qb", [128, 12], F32, kind="ExternalInput")
    vb_d = nc.dram_tensor("vb", [1, DIM], DT_BIG, kind="ExternalInput")
    pb_d = nc.dram_tensor("pb", [1, DIM], DT_BIG, kind="ExternalInput")
    onesr_d = nc.dram_tensor("onesr", [1, 128], DT_BIG, kind="ExternalInput")
    onesw_d = nc.dram_tensor("onesw", [128, 64], DT_ATT, kind="ExternalInput")
    out_d = nc.dram_tensor("out", [n_batches, N, DIM], F32, kind="ExternalOutput")

    with tile.TileContext(nc) as tc:
        with (
            tc.tile_pool(name="wpool", bufs=1) as wpool,
            tc.tile_pool(name="xpool", bufs=2) as xpool,
            tc.tile_pool(name="qkpool", bufs=2) as qkpool,
            tc.tile_pool(name="vpool", bufs=2) as vpool,
            tc.tile_pool(name="spool", bufs=3) as spool,
            tc.tile_pool(name="epool", bufs=3) as epool,
            tc.tile_pool(name="opool", bufs=2) as opool,
            tc.tile_pool(name="otpool", bufs=3) as otpool,
            tc.tile_pool(name="outpool", bufs=2) as outpool,
            tc.tile_pool(name="ps_mm", bufs=3, space="PSUM") as ps_mm,
            tc.tile_pool(name="ps_s", bufs=3, space="PSUM") as ps_s,
            tc.tile_pool(name="ps_pd", bufs=2, space="PSUM") as ps_pd,
        ):
            # ---- persistent weights ----
            qkw = []
            for ft in range(12):
                t = wpool.tile([128, KO, 128], DT_BIG, tag=f"qkw{ft}")
                nc.sync.dma_start(t[:], qkw_d[ft])
                qkw.append(t)
            vw = wpool.tile([128, KO, DIM], DT_BIG, tag="vw")
            nc.sync.dma_start(vw[:], vw_d[:])
            pw = wpool.tile([128, KO, DIM], DT_BIG, tag="pw")
            nc.sync.dma_start(pw[:], pw_d[:])
            bias = wpool.tile([128, 2, H, N], F32, tag="bias")
            for mo in range(2):
                nc.sync.dma_start(bias[:, mo], bias_d[mo])
            qb = wpool.tile([128, 12], F32, tag="qb")
            nc.sync.dma_start(qb[:], qb_d[:])
            vb = wpool.tile([1, DIM], DT_BIG, tag="vb")
            nc.sync.dma_start(vb[:], vb_d[:])
            pb = wpool.tile([1, DIM], DT_BIG, tag="pb")
            nc.sync.dma_start(pb[:], pb_d[:])
            ones_wide = wpool.tile([128, 64], DT_ATT, tag="ones_wide")
            nc.sync.dma_start(ones_wide[:], onesw_d[:])
            ones_row = wpool.tile([1, 128], DT_BIG, tag="ones_row")
            nc.sync.dma_start(ones_row[:], onesr_d[:])

            assert n_batches % 2 == 0
            for chunk in range(n_batches // 2):
                # ---- load x^T for a 2-batch chunk ----
                xt = xpool.tile([128, KO, 2 * N], DT_BIG, tag="xt")
                nc.sync.dma_start(xt[:], xt_d[:, :, 2 * N * chunk : 2 * N * (chunk + 1)])

                # ---- q^T / k^T (f-major), both batches at once (N=394) ----
                qkT = [
                    qkpool.tile([128, 12, N], DT_ATT, tag=f"qkT{i}", name=f"qkT{i}") for i in range(2)
                ]
                for ft in range(12):
                    ps = ps_mm.tile([128, 512], F32, tag="mm")
                    for ko in range(KO):
                        nc.tensor.matmul(
                            ps[:, 0 : 2 * N],
                            qkw[ft][:, ko],
                            xt[:, ko],
                            start=(ko == 0),
                            stop=(ko == KO - 1),
                        )
                    for i in range(2):
                        nc.scalar.activation(
                            qkT[i][:, ft],
                            ps[:, i * N : (i + 1) * N],
                            mybir.ActivationFunctionType.Identity,
                            bias=qb[:, ft : ft + 1],
                            scale=SCALE if ft < 6 else 1.0,
                        )

                for i in range(2):
                    b = 2 * chunk + i
                    boff = i * N

                    # ---- v (token-major) ----
                    v_sb = vpool.tile([128, 2, H, D], DT_ATT, tag="v")
                    for no, tw in ((0, N0), (1, N1)):
                        for fo, fw in ((0, 512), (512, 256)):
                            psv = ps_mm.tile([128, 512], F32, tag="mm")
                            for ko in range(KO):
                                nc.tensor.matmul(
                                    psv[0:tw, 0:fw],
                                    xt[:, ko, boff + no * 128 : boff + no * 128 + tw],
                                    vw[:, ko, fo : fo + fw],
                                    start=(ko == 0),
                                    stop=False,
                                )
                            nc.tensor.matmul(
                                psv[0:tw, 0:fw],
                                ones_row[0:1, 0:tw],
                                vb[0:1, fo : fo + fw],
                                start=False,
                                stop=True,
                            )
                            nh = fw // D
                            nc.vector.tensor_copy(
                                v_sb[0:tw, no, fo // D : fo // D + nh, :],
                                psv[0:tw, 0:fw].rearrange("p (a b) -> p a b", a=nh),
                            )

                    # ---- attention, head pairs (2j, 2j+1) ----
                    ohT = opool.tile([128, KO, N], DT_BIG, tag="ohT")
                    for j in range(H // 2):
                        es_pair = []
                        for hh in range(2):
                            h = 2 * j + hh
                            base = (h % 2) * 64
                            fq = h // 2
                            fk = 6 + h // 2
                            pss = ps_s.tile([128, 512], F32, tag="s")
                            kT = qkT[i][base : base + 64, fk, :]
                            qT = qkT[i][base : base + 64, fq, :]
                            nc.tensor.matmul(
                                pss[:, 0:N], kT[:, 0:128], qT, start=True, stop=True
                            )
                            nc.tensor.matmul(
                                pss[0:N1, N : 2 * N], kT[:, 128:N], qT, start=True, stop=True
                            )
                            # bias add (DVE, fp32) then exp (ACT, -> DT_ATT)
                            stmp = spool.tile([128, 2, N], F32, tag="stmp")
                            es = epool.tile([128, 2, N], DT_ATT, tag="es")
                            nc.vector.tensor_add(
                                stmp[:, 0, :], pss[:, 0:N], bias[:, 0, h, :]
                            )
                            nc.vector.tensor_add(
                                stmp[0:N1, 1, :], pss[0:N1, N : 2 * N], bias[0:N1, 1, h, :]
                            )
                            nc.scalar.activation(
                                es[:, 0, :], stmp[:, 0, :],
                                mybir.ActivationFunctionType.Exp,
                            )
                            nc.scalar.activation(
                                es[0:N1, 1, :], stmp[0:N1, 1, :],
                                mybir.ActivationFunctionType.Exp,
                            )
                            es_pair.append(es)

                        # stage 3: out^T[d, n] (cols 0:N) and denominators
                        # replicated across the 64 head dims (cols N:2N) so
                        # the division needs no partition broadcast.
                        pd = ps_pd.tile([128, 512], F32, tag="pd")
                        for hh in range(2):
                            h = 2 * j + hh
                            es = es_pair[hh]
                            rows = slice(hh * 64, hh * 64 + 64)
                            nc.tensor.matmul(
                                pd[rows, 0:N],
                                v_sb[:, 0, h, :],
                                es[:, 0, :],
                                start=True,
                                stop=False,
                            )
                            nc.tensor.matmul(
                                pd[rows, 0:N],
                                v_sb[0:N1, 1, h, :],
                                es[0:N1, 1, :],
                                start=False,
                                stop=True,
                            )
                            nc.tensor.matmul(
                                pd[rows, N : 2 * N],
                                ones_wide[:, :],
                                es[:, 0, :],
                                start=True,
                                stop=False,
                            )
                            nc.tensor.matmul(
                                pd[rows, N : 2 * N],
                                ones_wide[0:N1, :],
                                es[0:N1, 1, :],
                                start=False,
                                stop=True,
                            )

                        ot = otpool.tile([128, 2 * N], F32, tag="ot")
                        nc.scalar.activation(
                            ot[:], pd[:, 0 : 2 * N], mybir.ActivationFunctionType.Copy
                        )
                        rv = otpool.tile([128, N], F32, tag="rv")
                        nc.vector.reciprocal(rv[:], ot[:, N : 2 * N])
                        nc.vector.tensor_mul(ohT[:, j, :], ot[:, 0:N], rv[:])

                    # ---- projection (token-major out) + bias ----
                    out_sb = outpool.tile([128, 2, DIM], F32, tag="out")
                    for no, tw in ((0, N0), (1, N1)):
                        for fo, fw in ((0, 512), (512, 256)):
                            psp = ps_mm.tile([128, 512], F32, tag="mm")
                            for ko in range(KO):
                                nc.tensor.matmul(
                                    psp[0:tw, 0:fw],
                                    ohT[:, ko, no * 128 : no * 128 + tw],
                                    pw[:, ko, fo : fo + fw],
                                    start=(ko == 0),
                                    stop=False,
                                )
                            nc.tensor.matmul(
                                psp[0:tw, 0:fw],
                                ones_row[0:1, 0:tw],
                                pb[0:1, fo : fo + fw],
                                start=False,
                                stop=True,
                            )
                            nc.scalar.activation(
                                out_sb[0:tw, no, fo : fo + fw],
                                psp[0:tw, 0:fw],
                                mybir.ActivationFunctionType.Copy,
                            )
                    nc.sync.dma_start(out_d[b, 0:128, :], out_sb[:, 0, :])
                    nc.sync.dma_start(out_d[b, 128:N, :], out_sb[0:N1, 1, :])

    nc.compile()
    return nc


def _np_dt(dt):
    import ml_dtypes

    return {
        mybir.dt.float32: np.float32,
        mybir.dt.float32r: np.float32,
        mybir.dt.float16: np.float16,
        mybir.dt.bfloat16: ml_dtypes.bfloat16,
    }[dt]


def prep_inputs(x, qkv_w, q_bias, v_bias, rpb_table, proj_w, proj_b, rel_idx):
    """Host-side staging: shard x over cores, lay out weights for SBUF."""
    x = np.asarray(x, dtype=np.float32)
    qkv_w = np.asarray(qkv_w, dtype=np.float32)
    proj_w = np.asarray(proj_w, dtype=np.float32)
    q_bias = np.asarray(q_bias, dtype=np.float32)
    v_bias = np.asarray(v_bias, dtype=np.float32)
    rpb_table = np.asarray(rpb_table, dtype=np.float32)
    proj_b = np.asarray(proj_b, dtype=np.float32)
    rel_idx = np.asarray(rel_idx)

    big = _np_dt(DT_BIG)
    qkvwT = np.ascontiguousarray(qkv_w.T)  # [768, 2304]
    qkw = np.ascontiguousarray(
        qkvwT[:, : 2 * DIM].reshape(KO, 128, 12, 128).transpose(2, 1, 0, 3)
    ).astype(big)
    vw = np.ascontiguousarray(
        qkvwT[:, 2 * DIM :].reshape(KO, 128, DIM).transpose(1, 0, 2)
    ).astype(big)
    pw = np.ascontiguousarray(
        proj_w.T.reshape(KO, 128, DIM).transpose(1, 0, 2)
    ).astype(big)

    # bias[mo, mi, h, n] = rpb_table[rel_idx[n, m], h] with m = mo*128 + mi
    bnm = rpb_table[rel_idx]  # [n, m, H]
    bias = np.zeros((2 * 128, H, N), dtype=np.float32)
    bias[:N] = bnm.transpose(1, 2, 0)  # [m, H, n]
    bias = bias.reshape(2, 128, H, N)

    qb = np.zeros((128, 12), dtype=np.float32)
    qb[:, :6] = (q_bias * SCALE).reshape(KO, 128).T
    vb = np.ascontiguousarray(v_bias[None, :]).astype(big)
    pb = np.ascontiguousarray(proj_b[None, :]).astype(big)

    import ml_dtypes

    shared = {
        "qkw": qkw, "vw": vw, "pw": pw, "bias": np.ascontiguousarray(bias),
        "qb": qb, "vb": vb, "pb": pb,
        "onesr": np.ones((1, 128), dtype=_np_dt(DT_BIG)),
        "onesw": np.ones((128, 64), dtype=_np_dt(DT_ATT)),
    }
    in_maps = []
    for c in range(CORES):
        xs = x[c * BSH : (c + 1) * BSH]  # [BSH, N, DIM]
        xt = np.ascontiguousarray(
            xs.reshape(BSH * N, DIM).T.reshape(KO, 128, BSH * N).transpose(1, 0, 2)
        ).astype(big)
        in_maps.append({"xt": xt, **shared})
    return in_maps


def _ensure_ntff_hook():
    """Register the axon NTFF profile hook so trace=True yields exec_time_ns.

    The image's antenv package lacks axon_hooks, so boot() degrades silently;
    supply the module via sys.modules and re-register the ctypes hook.
    Best-effort: failure only disables tracing, not execution."""
    import types

    if "antenv.axon_hooks" in sys.modules:
        return
    try:
        mod = types.ModuleType("antenv.axon_hooks")
        _hook = [None]
        mod.set_axon_ntff_profile_hook = lambda h: _hook.__setitem__(0, h)
        mod.get_axon_ntff_profile_hook = lambda: _hook[0]
        from trn_agent_boot.trn_boot import _ntff_profile_via_ctypes

        mod.set_axon_ntff_profile_hook(
            _ntff_profile_via_ctypes("/opt/axon/libaxon_pjrt.so")
        )
        sys.modules["antenv.axon_hooks"] = mod
    except Exception:
        pass


_NC = None


def _get_nc():
    global _NC
    if _NC is None:
        _NC = build_program(BSH)
    return _NC


def kernel(x, qkv_w, q_bias, v_bias, rpb_table, proj_w, proj_b, rel_idx,
           _trace=False, **trace_kwargs):
    if _trace:
        _ensure_ntff_hook()
    nc = _get_nc()
    in_maps = prep_inputs(x, qkv_w, q_bias, v_bias, rpb_table, proj_w, proj_b, rel_idx)
    res = run_bass_kernel_spmd(
        nc, in_maps, core_ids=list(range(CORES)), trace=_trace, **trace_kwargs
    )
    out = np.concatenate([res.results[c]["out"] for c in range(CORES)], axis=0)
    if _trace:
        return out, res
    return out
